# revision 3
# baseline (speedup 1.0000x reference)
"""Trainium2 Bass kernel for nn_EnhancedObj (gnn_message_passing).

Per batch sample (data-parallel over 8 cores, one sample per core):
    ve  = LN(tanh(visual @ W_v + b_v))                  [64, 2048]
    oe  = LN(tanh(obj_flat @ W_o + b_o))                [2304, 2048]
    adj = softmax_n(oe @ ve^T / sqrt(2048))             [2304, 64]
    out = LN(tanh(adj^T @ oe + ve))                     [64, 2048]

Matmuls run in bf16 (fp32 PSUM accumulate).  The visual branch uses a
bf16 hi/lo split (3 matmuls) because its error propagates through the
softmax *and* the final residual; all other matmuls are plain bf16
(verified vs fp32 reference: maxabs ~1.6e-3, rel-fro ~2e-4).
"""

import numpy as np
import ml_dtypes

BF16 = ml_dtypes.bfloat16

BS = 8          # batch (== number of cores)
F = 64          # win_len (frames)
OBJ = 36        # objects per frame
D = 2048        # feature dim
N = F * OBJ     # 2304 objects per sample
NCH = N // 128  # 18 object-row chunks
KC = D // 128   # 16 contraction chunks
DW = 512        # matmul moving width (one PSUM bank of fp32)
ND = D // DW    # 4 output-column groups
LN_EPS = 1e-5

_BUILD_CACHE = {}


def _f32(x):
    return np.ascontiguousarray(np.asarray(x), dtype=np.float32)


def _klc_layout(w):
    """[D, M] -> [128(kl), KC*M] with element (kl, kc, m) = w[kc*128+kl, m]."""
    d, m = w.shape
    assert d == D
    return w.reshape(KC, 128, m).transpose(1, 0, 2).reshape(128, KC * m)


def _build(has_bv, has_bo, has_lnv, has_lno, has_lnov):
    """Build + compile the SPMD Bass program. Returns (nc, vec_names)."""
    key = (has_bv, has_bo, has_lnv, has_lno, has_lnov)
    if key in _BUILD_CACHE:
        return _BUILD_CACHE[key]

    import concourse.bass as bass
    import concourse.bacc as bacc
    import concourse.tile as tile
    from concourse import mybir

    f32 = mybir.dt.float32
    bf16 = mybir.dt.bfloat16
    AF = mybir.ActivationFunctionType
    AX = mybir.AxisListType
    OP = mybir.AluOpType

    general_B = has_bo or has_lno   # slower fp32 epilogue path in phase B

    nc = bacc.Bacc("TRN2", target_bir_lowering=False, debug=False, num_devices=BS)

    # ---- DRAM tensors -------------------------------------------------
    objT_d = nc.dram_tensor("objT", [NCH, 128, KC * 128], bf16, kind="ExternalInput").ap()
    wo_d = nc.dram_tensor("Wo", [128, KC * D], bf16, kind="ExternalInput").ap()
    wvh_d = nc.dram_tensor("Wv_hi", [128, KC * D], bf16, kind="ExternalInput").ap()
    wvl_d = nc.dram_tensor("Wv_lo", [128, KC * D], bf16, kind="ExternalInput").ap()
    vth_d = nc.dram_tensor("vT_hi", [128, KC * F], bf16, kind="ExternalInput").ap()
    vtl_d = nc.dram_tensor("vT_lo", [128, KC * F], bf16, kind="ExternalInput").ap()
    vec_names = []
    vec_d = {}
    for name, used in [
        ("b_v", has_bv), ("b_o", has_bo),
        ("ln_v_g", has_lnv), ("ln_v_b", has_lnv),
        ("ln_o_g", has_lno), ("ln_o_b", has_lno),
        ("ln_ov_g", has_lnov), ("ln_ov_b", has_lnov),
    ]:
        if used:
            vec_d[name] = nc.dram_tensor(name, [D], f32, kind="ExternalInput").ap()
            vec_names.append(name)
    out_d = nc.dram_tensor("out", [F, D], f32, kind="ExternalOutput").ap()

    def bcast_row(vec_ap, parts):
        """AP reading a [D] DRAM vector broadcast over `parts` partitions."""
        return bass.AP(
            tensor=vec_ap.tensor,
            offset=vec_ap.offset,
            ap=[[0, parts]] + list(vec_ap.ap),
        )

    inv_sqrt_d = 1.0 / float(np.sqrt(D))

    with tile.TileContext(nc) as tc:
        with tc.tile_pool(name="persist", bufs=1) as persist, \
             tc.tile_pool(name="stats", bufs=2) as stats_pool:

            eps128 = persist.tile([128, 1], f32)
            nc.vector.memset(eps128, LN_EPS)

            # broadcast bias/gain tiles (only in the general path)
            bc = {}
            for name in vec_names:
                parts = 128 if name in ("b_o", "ln_o_g", "ln_o_b") else F
                t = persist.tile([parts, D], f32, name=f"bc_{name}", tag=f"bc_{name}")
                nc.gpsimd.dma_start(out=t, in_=bcast_row(vec_d[name], parts))
                bc[name] = t

            def layer_norm_stats(t_in, rows):
                """mean/var over free dim -> mvr tile [:,0]=mu [:,2]=rstd."""
                st = stats_pool.tile([128, ND, nc.vector.BN_STATS_DIM], f32, tag="st")
                for j in range(ND):
                    nc.vector.bn_stats(out=st[:rows, j, :],
                                       in_=t_in[:rows, j * DW:(j + 1) * DW])
                mvr = stats_pool.tile([128, 3], f32, tag="mvr")
                nc.vector.bn_aggr(out=mvr[:rows, 0:2], in_=st[:rows])
                nc.scalar.activation(out=mvr[:rows, 2:3], in_=mvr[:rows, 1:2],
                                     func=AF.Sqrt, bias=eps128[:rows], scale=1.0)
                nc.vector.reciprocal(out=mvr[:rows, 2:3], in_=mvr[:rows, 2:3])
                return mvr

            def apply_ln(t_in, rows, out_tile, mvr, gname, has_g, tn_tile=None):
                """out = (t_in - mu) * rstd [* g + b]; out may cast dtype."""
                if has_g:
                    nc.vector.tensor_scalar(
                        out=tn_tile[:rows], in0=t_in[:rows],
                        scalar1=mvr[:rows, 0:1], scalar2=mvr[:rows, 2:3],
                        op0=OP.subtract, op1=OP.mult)
                    nc.vector.tensor_mul(out=tn_tile[:rows], in0=tn_tile[:rows],
                                         in1=bc[gname + "_g"][:rows])
                    nc.vector.tensor_add(out=out_tile[:rows], in0=tn_tile[:rows],
                                         in1=bc[gname + "_b"][:rows])
                else:
                    nc.vector.tensor_scalar(
                        out=out_tile[:rows], in0=t_in[:rows],
                        scalar1=mvr[:rows, 0:1], scalar2=mvr[:rows, 2:3],
                        op0=OP.subtract, op1=OP.mult)

            # ======== Phase A: visual branch (bf16 hi/lo x3) ===========
            ve_nat = persist.tile([F, D], f32)          # LN'd visual embedding
            veT = persist.tile([128, KC, F], bf16)      # transposed, for phase C

            with tc.tile_pool(name="wv", bufs=2) as wvp, \
                 tc.tile_pool(name="vt", bufs=1) as vtp, \
                 tc.tile_pool(name="psA", bufs=1, space="PSUM") as psA, \
                 tc.tile_pool(name="tmpA", bufs=1) as tmpA:
                vth = vtp.tile([128, KC, F], bf16)
                vtl = vtp.tile([128, KC, F], bf16)
                nc.sync.dma_start(out=vth, in_=vth_d)
                nc.sync.dma_start(out=vtl, in_=vtl_d)

                ps_ve = psA.tile([F, D], f32)
                n_pairs = 3
                for kc in range(KC):
                    wvh_k = wvp.tile([128, D], bf16, tag="wvh")
                    wvl_k = wvp.tile([128, D], bf16, tag="wvl")
                    nc.sync.dma_start(out=wvh_k, in_=wvh_d[:, kc * D:(kc + 1) * D])
                    nc.sync.dma_start(out=wvl_k, in_=wvl_d[:, kc * D:(kc + 1) * D])
                    pairs = [(vth, wvh_k), (vth, wvl_k), (vtl, wvh_k)]
                    for pi, (lt, rt) in enumerate(pairs):
                        for dd in range(ND):
                            nc.tensor.matmul(
                                ps_ve[:, dd * DW:(dd + 1) * DW],
                                lhsT=lt[:, kc, :],
                                rhs=rt[:, dd * DW:(dd + 1) * DW],
                                start=(kc == 0 and pi == 0),
                                stop=(kc == KC - 1 and pi == n_pairs - 1))

                tA = tmpA.tile([F, D], f32)
                if has_bv:
                    nc.vector.tensor_add(out=tA, in0=ps_ve, in1=bc["b_v"])
                    nc.scalar.activation(out=tA, in_=tA, func=AF.Tanh)
                else:
                    nc.scalar.activation(out=tA, in_=ps_ve, func=AF.Tanh)
                mvr = layer_norm_stats(tA, F)
                tnA = tmpA.tile([F, D], f32) if has_lnv else None
                apply_ln(tA, F, ve_nat, mvr, "ln_v", has_lnv, tnA)

                ve_bf = tmpA.tile([F, D], bf16)
                nc.vector.tensor_copy(out=ve_bf, in_=ve_nat)
                # [64, 2048] -> rows d=(kc*128+kl): [kl, kc, f]
                nc.sync.dma_start(out=veT, in_=ve_bf, transpose=True)

            # ======== Phase B: object branch (the big matmul) ==========
            oe_nat = persist.tile([128, NCH, D], bf16)  # LN'd object embeddings

            with tc.tile_pool(name="wo", bufs=1) as wop, \
                 tc.tile_pool(name="objs", bufs=1 if general_B else 2) as objp, \
                 tc.tile_pool(name="psB", bufs=2, space="PSUM") as psB, \
                 tc.tile_pool(name="tmpB", bufs=1 if general_B else 2) as tmpB:
                wo = wop.tile([128, KC * D], bf16)
                nc.sync.dma_start(out=wo, in_=wo_d)

                for nch in range(NCH):
                    objT_nc = objp.tile([128, KC, 128], bf16, tag="objT")
                    nc.sync.dma_start(out=objT_nc, in_=objT_d[nch])
                    ps = psB.tile([128, D], f32, tag="psb")
                    for kc in range(KC):
                        for dd in range(ND):
                            nc.tensor.matmul(
                                ps[:, dd * DW:(dd + 1) * DW],
                                lhsT=objT_nc[:, kc, :],
                                rhs=wo[:, kc * D + dd * DW: kc * D + (dd + 1) * DW],
                                start=(kc == 0), stop=(kc == KC - 1))
                    tB = tmpB.tile([128, D], f32 if general_B else bf16, tag="tB")
                    if has_bo:
                        nc.vector.tensor_add(out=tB, in0=ps, in1=bc["b_o"])
                        nc.scalar.activation(out=tB, in_=tB, func=AF.Tanh)
                    else:
                        nc.scalar.activation(out=tB, in_=ps, func=AF.Tanh)
                    mvr = layer_norm_stats(tB, 128)
                    tnB = tmpB.tile([128, D], f32, tag="tnB") if has_lno else None
                    apply_ln(tB, 128, oe_nat[:, nch, :], mvr, "ln_o", has_lno, tnB)

            # ======== Phase C: adjacency + softmax =====================
            p_nat = persist.tile([128, NCH, F], bf16)   # softmax probs, natural

            with tc.tile_pool(name="oeT", bufs=2) as oetp, \
                 tc.tile_pool(name="psC", bufs=2, space="PSUM") as psC, \
                 tc.tile_pool(name="tmpC", bufs=1) as tmpC:
                logits = tmpC.tile([F, N], f32)

                n_slices = []   # (start_block, n_blocks)
                nb = 0
                while nb < NCH:
                    w = min(4, NCH - nb)
                    n_slices.append((nb, w))
                    nb += w

                for b0, bw in n_slices:
                    win = oetp.tile([128, 4, KC, 128], bf16, tag="oeTwin")
                    for j in range(bw):
                        # [128(nw), 2048(d)] -> rows d=(kc*128+kl): [kl, kc, nw]
                        nc.sync.dma_start(out=win[:, j, :, :],
                                          in_=oe_nat[:, b0 + j, :], transpose=True)
                    ps = psC.tile([F, DW], f32, tag="padj")
                    for kc in range(KC):
                        nc.tensor.matmul(
                            ps[:, :bw * 128],
                            lhsT=veT[:, kc, :],
                            rhs=win[:, :bw, kc, :],
                            start=(kc == 0), stop=(kc == KC - 1))
                    nc.scalar.activation(out=logits[:, b0 * 128:(b0 + bw) * 128],
                                         in_=ps[:, :bw * 128],
                                         func=AF.Copy, scale=inv_sqrt_d)

                red = tmpC.tile([F, 2], f32)
                nc.vector.reduce_max(out=red[:, 0:1], in_=logits, axis=AX.X,
                                     negate=True)
                nc.scalar.activation(out=logits, in_=logits, func=AF.Exp,
                                     bias=red[:, 0:1], scale=1.0)
                nc.vector.reduce_sum(out=red[:, 1:2], in_=logits, axis=AX.X)
                nc.vector.reciprocal(out=red[:, 1:2], in_=red[:, 1:2])
                p_bf = tmpC.tile([F, N], bf16)
                nc.vector.tensor_scalar_mul(out=p_bf, in0=logits, scalar1=red[:, 1:2])
                # [64, 2304] -> rows n=(ncb*128+nw): [nw, ncb, f]
                nc.sync.dma_start(out=p_nat, in_=p_bf, transpose=True)

            # ======== Phase D: aggregate + residual + LN ===============
            with tc.tile_pool(name="psD", bufs=1, space="PSUM") as psD, \
                 tc.tile_pool(name="tmpD", bufs=1) as tmpD:
                ps_agg = psD.tile([F, D], f32)
                for nch in range(NCH):
                    for dd in range(ND):
                        nc.tensor.matmul(
                            ps_agg[:, dd * DW:(dd + 1) * DW],
                            lhsT=p_nat[:, nch, :],
                            rhs=oe_nat[:, nch, dd * DW:(dd + 1) * DW],
                            start=(nch == 0), stop=(nch == NCH - 1))

                tD = tmpD.tile([F, D], f32)
                nc.vector.tensor_add(out=tD, in0=ps_agg, in1=ve_nat)
                nc.scalar.activation(out=tD, in_=tD, func=AF.Tanh)
                out_f = tmpD.tile([F, D], f32)
                mvr = layer_norm_stats(tD, F)
                tnD = tmpD.tile([F, D], f32) if has_lnov else None
                apply_ln(tD, F, out_f, mvr, "ln_ov", has_lnov, tnD)
                nc.sync.dma_start(out=out_d, in_=out_f)

    nc.compile()
    _BUILD_CACHE[key] = (nc, vec_names)
    return nc, vec_names


def _prep_core_inputs(visual, obj_flat, shared):
    """Host-side per-sample layout prep. visual [64,2048] f32, obj_flat [2304,2048] f32."""
    vT = np.ascontiguousarray(visual.T)                     # [2048, 64] f32
    vth = vT.astype(BF16)
    vtl = (vT - vth.astype(np.float32)).astype(BF16)
    m = {
        "objT": np.ascontiguousarray(
            obj_flat.reshape(NCH, 128, KC, 128).transpose(0, 3, 2, 1)
        ).astype(BF16).reshape(NCH, 128, KC * 128),
        "vT_hi": np.ascontiguousarray(_klc_layout(vth.astype(np.float32)).astype(BF16)),
        "vT_lo": np.ascontiguousarray(_klc_layout(vtl.astype(np.float32)).astype(BF16)),
    }
    m.update(shared)
    return m


def run_kernel(inputs, trace=False):
    """Returns (out [8, 64, 2048] fp32, exec_time_ns or None)."""
    from concourse import bass_utils

    visual = _f32(inputs["visual_feats"])            # [8, 64, 2048]
    obj = _f32(inputs["obj_feats"])                  # [8, 64, 36, 2048]
    W_v = _f32(inputs["W_v"])
    W_o = _f32(inputs["W_o"])
    vecs = {k: _f32(inputs[k]) for k in
            ["b_v", "b_o", "ln_v_g", "ln_v_b", "ln_o_g", "ln_o_b", "ln_ov_g", "ln_ov_b"]}

    has_bv = not np.all(vecs["b_v"] == 0)
    has_bo = not np.all(vecs["b_o"] == 0)
    has_lnv = not (np.all(vecs["ln_v_g"] == 1) and np.all(vecs["ln_v_b"] == 0))
    has_lno = not (np.all(vecs["ln_o_g"] == 1) and np.all(vecs["ln_o_b"] == 0))
    has_lnov = not (np.all(vecs["ln_ov_g"] == 1) and np.all(vecs["ln_ov_b"] == 0))

    nc, vec_names = _build(has_bv, has_bo, has_lnv, has_lno, has_lnov)

    wvh = W_v.astype(BF16)
    wvl = (W_v - wvh.astype(np.float32)).astype(BF16)
    shared = {
        "Wo": np.ascontiguousarray(
            _klc_layout(W_o.astype(BF16).astype(np.float32)).astype(BF16)),
        "Wv_hi": np.ascontiguousarray(_klc_layout(wvh.astype(np.float32)).astype(BF16)),
        "Wv_lo": np.ascontiguousarray(_klc_layout(wvl.astype(np.float32)).astype(BF16)),
    }
    for name in vec_names:
        shared[name] = vecs[name]

    in_maps = [
        _prep_core_inputs(visual[c], obj[c].reshape(N, D), shared)
        for c in range(BS)
    ]

    res = bass_utils.run_bass_kernel_spmd(
        nc, in_maps, core_ids=list(range(BS)), trace=trace)
    out = np.stack([res.results[c]["out"] for c in range(BS)], axis=0)
    return out.astype(np.float32), res.exec_time_ns


def kernel(**inputs):
    out, _ = run_kernel(inputs, trace=False)
    return out


# revision 4
# speedup vs baseline: 1.1321x; 1.1321x over previous
"""Trainium2 Bass kernel for nn_EnhancedObj (gnn_message_passing).

Per batch sample (data-parallel over 8 cores, one sample per core):
    ve  = LN(tanh(visual @ W_v + b_v))                  [64, 2048]
    oe  = LN(tanh(obj_flat @ W_o + b_o))                [2304, 2048]
    adj = softmax_n(oe @ ve^T / sqrt(2048))             [2304, 64]
    out = LN(tanh(adj^T @ oe + ve))                     [64, 2048]

All matmuls run in fp16 (fp32 PSUM accumulate) — fp16 streams at the
same 1 col/cycle as bf16 on the TRN2 PE but carries a 10-bit mantissa
(verified vs fp32 reference: maxabs ~3e-3 on a ~1.4-absmax output,
rel-fro ~3.6e-4).  Softmax and all LayerNorm statistics are fp32.
"""

import numpy as np

F16 = np.float16

BS = 8          # batch (== number of cores)
F = 64          # win_len (frames)
OBJ = 36        # objects per frame
D = 2048        # feature dim
N = F * OBJ     # 2304 objects per sample
NCH = N // 128  # 18 object-row chunks
KC = D // 128   # 16 contraction chunks
DW = 512        # matmul moving width (one PSUM bank of fp32)
ND = D // DW    # 4 output-column groups
LN_EPS = 1e-5

_BUILD_CACHE = {}


def _f32(x):
    return np.ascontiguousarray(np.asarray(x), dtype=np.float32)


def _klc_layout(w):
    """[D, M] -> [128(kl), KC*M] with element (kl, kc, m) = w[kc*128+kl, m]."""
    d, m = w.shape
    assert d == D
    return w.reshape(KC, 128, m).transpose(1, 0, 2).reshape(128, KC * m)


def _build(has_bv, has_bo, has_lnv, has_lno, has_lnov):
    """Build + compile the SPMD Bass program. Returns (nc, vec_names)."""
    key = (has_bv, has_bo, has_lnv, has_lno, has_lnov)
    if key in _BUILD_CACHE:
        return _BUILD_CACHE[key]

    import concourse.bass as bass
    import concourse.bacc as bacc
    import concourse.tile as tile
    from concourse import mybir

    f32 = mybir.dt.float32
    f16 = mybir.dt.float16
    AF = mybir.ActivationFunctionType
    AX = mybir.AxisListType
    OP = mybir.AluOpType

    general_B = has_bo or has_lno   # slower fp32 epilogue path in phase B

    nc = bacc.Bacc("TRN2", target_bir_lowering=False, debug=False, num_devices=BS)

    # ---- DRAM tensors -------------------------------------------------
    objT_d = nc.dram_tensor("objT", [NCH, 128, KC * 128], f16, kind="ExternalInput").ap()
    wo_d = nc.dram_tensor("Wo", [128, KC * D], f16, kind="ExternalInput").ap()
    wv_d = nc.dram_tensor("Wv", [128, KC * D], f16, kind="ExternalInput").ap()
    vt_d = nc.dram_tensor("vT", [128, KC * F], f16, kind="ExternalInput").ap()
    vec_names = []
    vec_d = {}
    for name, used in [
        ("b_v", has_bv), ("b_o", has_bo),
        ("ln_v_g", has_lnv), ("ln_v_b", has_lnv),
        ("ln_o_g", has_lno), ("ln_o_b", has_lno),
        ("ln_ov_g", has_lnov), ("ln_ov_b", has_lnov),
    ]:
        if used:
            vec_d[name] = nc.dram_tensor(name, [D], f32, kind="ExternalInput").ap()
            vec_names.append(name)
    out_d = nc.dram_tensor("out", [F, D], f32, kind="ExternalOutput").ap()

    def bcast_row(vec_ap, parts):
        """AP reading a [D] DRAM vector broadcast over `parts` partitions."""
        return bass.AP(
            tensor=vec_ap.tensor,
            offset=vec_ap.offset,
            ap=[[0, parts]] + list(vec_ap.ap),
        )

    inv_sqrt_d = 1.0 / float(np.sqrt(D))

    with tile.TileContext(nc) as tc:
        with tc.tile_pool(name="persist", bufs=1) as persist, \
             tc.tile_pool(name="stats", bufs=1 if general_B else 2) as stats_pool:

            eps128 = persist.tile([128, 1], f32)
            nc.vector.memset(eps128, LN_EPS)

            # broadcast bias/gain tiles (only in the general path)
            bc = {}
            for name in vec_names:
                parts = 128 if name in ("b_o", "ln_o_g", "ln_o_b") else F
                t = persist.tile([parts, D], f32, name=f"bc_{name}", tag=f"bc_{name}")
                nc.gpsimd.dma_start(out=t, in_=bcast_row(vec_d[name], parts))
                bc[name] = t

            def layer_norm_stats(t_in, rows):
                """mean/var over free dim -> mvr tile [:,0]=mu [:,2]=rstd."""
                st = stats_pool.tile([128, ND, nc.vector.BN_STATS_DIM], f32, tag="st")
                for j in range(ND):
                    nc.vector.bn_stats(out=st[:rows, j, :],
                                       in_=t_in[:rows, j * DW:(j + 1) * DW])
                mvr = stats_pool.tile([128, 3], f32, tag="mvr")
                nc.vector.bn_aggr(out=mvr[:rows, 0:2], in_=st[:rows])
                nc.scalar.activation(out=mvr[:rows, 2:3], in_=mvr[:rows, 1:2],
                                     func=AF.Sqrt, bias=eps128[:rows], scale=1.0)
                nc.vector.reciprocal(out=mvr[:rows, 2:3], in_=mvr[:rows, 2:3])
                return mvr

            def apply_ln(t_in, rows, out_tile, mvr, gname, has_g, tn_tile=None):
                """out = (t_in - mu) * rstd [* g + b]; out may cast dtype."""
                if has_g:
                    nc.vector.tensor_scalar(
                        out=tn_tile[:rows], in0=t_in[:rows],
                        scalar1=mvr[:rows, 0:1], scalar2=mvr[:rows, 2:3],
                        op0=OP.subtract, op1=OP.mult)
                    nc.vector.tensor_mul(out=tn_tile[:rows], in0=tn_tile[:rows],
                                         in1=bc[gname + "_g"][:rows])
                    nc.vector.tensor_add(out=out_tile[:rows], in0=tn_tile[:rows],
                                         in1=bc[gname + "_b"][:rows])
                else:
                    nc.vector.tensor_scalar(
                        out=out_tile[:rows], in0=t_in[:rows],
                        scalar1=mvr[:rows, 0:1], scalar2=mvr[:rows, 2:3],
                        op0=OP.subtract, op1=OP.mult)

            ve_nat = persist.tile([F, D], f32)          # LN'd visual embedding
            veT = persist.tile([128, KC, F], f16)       # transposed, for phase C
            oe_nat = persist.tile([128, NCH, D], f16)   # LN'd object embeddings
            p_nat = persist.tile([128, NCH, F], f16)    # softmax probs, natural

            # W_o spans phases A+B so its DMA overlaps phase A compute.
            with tc.tile_pool(name="wo", bufs=1) as wop:
                wo = wop.tile([128, KC * D], f16)
                nc.sync.dma_start(out=wo, in_=wo_d)

                # ==== Phase A: visual branch (single fp16 pass) ========
                with tc.tile_pool(name="wv", bufs=4) as wvp, \
                     tc.tile_pool(name="vt", bufs=1) as vtp, \
                     tc.tile_pool(name="psA", bufs=1, space="PSUM") as psA, \
                     tc.tile_pool(name="tmpA", bufs=1) as tmpA:
                    vt = vtp.tile([128, KC, F], f16)
                    nc.sync.dma_start(out=vt, in_=vt_d)

                    ps_ve = psA.tile([F, D], f32)
                    for kc in range(KC):
                        wv_k = wvp.tile([128, D], f16, tag="wvk")
                        nc.sync.dma_start(out=wv_k, in_=wv_d[:, kc * D:(kc + 1) * D])
                        for dd in range(ND):
                            nc.tensor.matmul(
                                ps_ve[:, dd * DW:(dd + 1) * DW],
                                lhsT=vt[:, kc, :],
                                rhs=wv_k[:, dd * DW:(dd + 1) * DW],
                                start=(kc == 0), stop=(kc == KC - 1))

                    tA = tmpA.tile([F, D], f32)
                    if has_bv:
                        nc.vector.tensor_add(out=tA, in0=ps_ve, in1=bc["b_v"])
                        nc.scalar.activation(out=tA, in_=tA, func=AF.Tanh)
                    else:
                        nc.scalar.activation(out=tA, in_=ps_ve, func=AF.Tanh)
                    mvr = layer_norm_stats(tA, F)
                    tnA = tmpA.tile([F, D], f32) if has_lnv else None
                    apply_ln(tA, F, ve_nat, mvr, "ln_v", has_lnv, tnA)

                    ve_bf = tmpA.tile([F, D], f16)
                    nc.vector.tensor_copy(out=ve_bf, in_=ve_nat)
                    # [64, 2048] -> rows d=(kc*128+kl): [kl, kc, f]
                    nc.sync.dma_start(out=veT, in_=ve_bf, transpose=True)

                # ==== Phase B: object branch (the big matmul) ==========
                with tc.tile_pool(name="objs", bufs=1 if general_B else 2) as objp, \
                     tc.tile_pool(name="psB", bufs=2, space="PSUM") as psB, \
                     tc.tile_pool(name="tmpB", bufs=1 if general_B else 2) as tmpB:
                    for nch in range(NCH):
                        objT_nc = objp.tile([128, KC, 128], f16, tag="objT")
                        nc.sync.dma_start(out=objT_nc, in_=objT_d[nch])
                        ps = psB.tile([128, D], f32, tag="psb")
                        for kc in range(KC):
                            for dd in range(ND):
                                nc.tensor.matmul(
                                    ps[:, dd * DW:(dd + 1) * DW],
                                    lhsT=objT_nc[:, kc, :],
                                    rhs=wo[:, kc * D + dd * DW: kc * D + (dd + 1) * DW],
                                    start=(kc == 0), stop=(kc == KC - 1))
                        tB = tmpB.tile([128, D], f32 if general_B else f16, tag="tB")
                        if has_bo:
                            nc.vector.tensor_add(out=tB, in0=ps, in1=bc["b_o"])
                            nc.scalar.activation(out=tB, in_=tB, func=AF.Tanh)
                        else:
                            nc.scalar.activation(out=tB, in_=ps, func=AF.Tanh)
                        mvr = layer_norm_stats(tB, 128)
                        tnB = tmpB.tile([128, D], f32, tag="tnB") if has_lno else None
                        apply_ln(tB, 128, oe_nat[:, nch, :], mvr, "ln_o", has_lno, tnB)

            # ==== Phases C+D: adjacency softmax, aggregate, output =====
            with tc.tile_pool(name="oeT", bufs=1 if general_B else 2) as oetp, \
                 tc.tile_pool(name="psC", bufs=2, space="PSUM") as psC, \
                 tc.tile_pool(name="psD", bufs=1, space="PSUM") as psD, \
                 tc.tile_pool(name="tmpC", bufs=1) as tmpC, \
                 tc.tile_pool(name="tmpD", bufs=1) as tmpD:
                logits = tmpC.tile([F, N], f32)

                n_slices = []   # (start_block, n_blocks)
                nb = 0
                while nb < NCH:
                    w = min(4, NCH - nb)
                    n_slices.append((nb, w))
                    nb += w

                for b0, bw in n_slices:
                    win = oetp.tile([128, 4, KC, 128], f16, tag="oeTwin")
                    for j in range(bw):
                        # [128(nw), 2048(d)] -> rows d=(kc*128+kl): [kl, kc, nw]
                        nc.sync.dma_start(out=win[:, j, :, :],
                                          in_=oe_nat[:, b0 + j, :], transpose=True)
                    ps = psC.tile([F, DW], f32, tag="padj")
                    for kc in range(KC):
                        nc.tensor.matmul(
                            ps[:, :bw * 128],
                            lhsT=veT[:, kc, :],
                            rhs=win[:, :bw, kc, :],
                            start=(kc == 0), stop=(kc == KC - 1))
                    nc.scalar.activation(out=logits[:, b0 * 128:(b0 + bw) * 128],
                                         in_=ps[:, :bw * 128],
                                         func=AF.Copy, scale=inv_sqrt_d)

                red = tmpC.tile([F, 2], f32)
                nc.vector.reduce_max(out=red[:, 0:1], in_=logits, axis=AX.X,
                                     negate=True)
                nc.scalar.activation(out=logits, in_=logits, func=AF.Exp,
                                     bias=red[:, 0:1], scale=1.0)
                nc.vector.reduce_sum(out=red[:, 1:2], in_=logits, axis=AX.X)
                nc.vector.reciprocal(out=red[:, 1:2], in_=red[:, 1:2])
                p_bf = tmpC.tile([F, N], f16)
                nc.vector.tensor_scalar_mul(out=p_bf, in0=logits, scalar1=red[:, 1:2])
                # [64, 2304] -> rows n=(ncb*128+nw): [nw, ncb, f]
                nc.sync.dma_start(out=p_nat, in_=p_bf, transpose=True)

                ps_agg = psD.tile([F, D], f32)
                for nch in range(NCH):
                    for dd in range(ND):
                        nc.tensor.matmul(
                            ps_agg[:, dd * DW:(dd + 1) * DW],
                            lhsT=p_nat[:, nch, :],
                            rhs=oe_nat[:, nch, dd * DW:(dd + 1) * DW],
                            start=(nch == 0), stop=(nch == NCH - 1))

                tD = tmpD.tile([F, D], f32)
                nc.vector.tensor_add(out=tD, in0=ps_agg, in1=ve_nat)
                nc.scalar.activation(out=tD, in_=tD, func=AF.Tanh)
                out_f = tmpD.tile([F, D], f32)
                mvr = layer_norm_stats(tD, F)
                tnD = tmpD.tile([F, D], f32) if has_lnov else None
                apply_ln(tD, F, out_f, mvr, "ln_ov", has_lnov, tnD)
                nc.sync.dma_start(out=out_d, in_=out_f)

    nc.compile()
    _BUILD_CACHE[key] = (nc, vec_names)
    return nc, vec_names


def _prep_core_inputs(visual, obj_flat, shared):
    """Host-side per-sample layout prep. visual [64,2048] f32, obj_flat [2304,2048] f32."""
    m = {
        "objT": np.ascontiguousarray(
            obj_flat.reshape(NCH, 128, KC, 128).transpose(0, 3, 2, 1)
        ).astype(F16).reshape(NCH, 128, KC * 128),
        "vT": np.ascontiguousarray(
            _klc_layout(np.ascontiguousarray(visual.T))).astype(F16),
    }
    m.update(shared)
    return m


def run_kernel(inputs, trace=False):
    """Returns (out [8, 64, 2048] fp32, exec_time_ns or None)."""
    from concourse import bass_utils

    visual = _f32(inputs["visual_feats"])            # [8, 64, 2048]
    obj = _f32(inputs["obj_feats"])                  # [8, 64, 36, 2048]
    W_v = _f32(inputs["W_v"])
    W_o = _f32(inputs["W_o"])
    vecs = {k: _f32(inputs[k]) for k in
            ["b_v", "b_o", "ln_v_g", "ln_v_b", "ln_o_g", "ln_o_b", "ln_ov_g", "ln_ov_b"]}

    has_bv = not np.all(vecs["b_v"] == 0)
    has_bo = not np.all(vecs["b_o"] == 0)
    has_lnv = not (np.all(vecs["ln_v_g"] == 1) and np.all(vecs["ln_v_b"] == 0))
    has_lno = not (np.all(vecs["ln_o_g"] == 1) and np.all(vecs["ln_o_b"] == 0))
    has_lnov = not (np.all(vecs["ln_ov_g"] == 1) and np.all(vecs["ln_ov_b"] == 0))

    nc, vec_names = _build(has_bv, has_bo, has_lnv, has_lno, has_lnov)

    shared = {
        "Wo": np.ascontiguousarray(_klc_layout(W_o)).astype(F16),
        "Wv": np.ascontiguousarray(_klc_layout(W_v)).astype(F16),
    }
    for name in vec_names:
        shared[name] = vecs[name]

    in_maps = [
        _prep_core_inputs(visual[c], obj[c].reshape(N, D), shared)
        for c in range(BS)
    ]

    res = bass_utils.run_bass_kernel_spmd(
        nc, in_maps, core_ids=list(range(BS)), trace=trace)
    out = np.stack([res.results[c]["out"] for c in range(BS)], axis=0)
    return out.astype(np.float32), res.exec_time_ns


def kernel(**inputs):
    out, _ = run_kernel(inputs, trace=False)
    return out


# revision 5
# speedup vs baseline: 1.2140x; 1.0723x over previous
"""Trainium2 Bass kernel for nn_EnhancedObj (gnn_message_passing).

Per batch sample (data-parallel over 8 cores, one sample per core):
    ve  = LN(tanh(visual @ W_v + b_v))                  [64, 2048]
    oe  = LN(tanh(obj_flat @ W_o + b_o))                [2304, 2048]
    adj = softmax_n(oe @ ve^T / sqrt(2048))             [2304, 64]
    out = LN(tanh(adj^T @ oe + ve))                     [64, 2048]

All matmuls run in fp16 (fp32 PSUM accumulate) — fp16 streams at the
same 1 col/cycle as bf16 on the TRN2 PE but carries a 10-bit mantissa
(verified vs fp32 reference: maxabs ~3e-3 on a ~1.4-absmax output,
rel-fro ~3.6e-4).  Softmax and all LayerNorm statistics are fp32.

Schedule: the adjacency matmuls (phase C) are interleaved into the big
object-branch matmul stream (phase B) two object-chunks at a time, so
the oe transposes ride B's otherwise-idle DMA and the PE never waits
for them.  B uses 3x[128,1024] PSUM tiles (6 banks), leaving 2 banks
for the adjacency accumulation.
"""

import numpy as np

F16 = np.float16

BS = 8          # batch (== number of cores)
F = 64          # win_len (frames)
OBJ = 36        # objects per frame
D = 2048        # feature dim
N = F * OBJ     # 2304 objects per sample
NCH = N // 128  # 18 object-row chunks
NW = NCH // 2   # 9 two-chunk adjacency windows
KC = D // 128   # 16 contraction chunks
DW = 512        # matmul moving width (one PSUM bank of fp32)
ND = D // DW    # 4 output-column groups
LN_EPS = 1e-5

_BUILD_CACHE = {}


def _f32(x):
    return np.ascontiguousarray(np.asarray(x), dtype=np.float32)


def _klc_layout(w):
    """[D, M] -> [128(kl), KC*M] with element (kl, kc, m) = w[kc*128+kl, m]."""
    d, m = w.shape
    assert d == D
    return w.reshape(KC, 128, m).transpose(1, 0, 2).reshape(128, KC * m)


def _build(has_bv, has_bo, has_lnv, has_lno, has_lnov):
    """Build + compile the SPMD Bass program. Returns (nc, vec_names)."""
    key = (has_bv, has_bo, has_lnv, has_lno, has_lnov)
    if key in _BUILD_CACHE:
        return _BUILD_CACHE[key]

    import concourse.bass as bass
    import concourse.bacc as bacc
    import concourse.tile as tile
    from concourse import mybir

    f32 = mybir.dt.float32
    f16 = mybir.dt.float16
    AF = mybir.ActivationFunctionType
    AX = mybir.AxisListType
    OP = mybir.AluOpType

    general_B = has_bo or has_lno   # slower fp32 epilogue path in phase B

    nc = bacc.Bacc("TRN2", target_bir_lowering=False, debug=False, num_devices=BS)

    # ---- DRAM tensors -------------------------------------------------
    objT_d = nc.dram_tensor("objT", [NCH, 128, KC * 128], f16, kind="ExternalInput").ap()
    wo_d = nc.dram_tensor("Wo", [128, KC * D], f16, kind="ExternalInput").ap()
    wv_d = nc.dram_tensor("Wv", [128, KC * D], f16, kind="ExternalInput").ap()
    vt_d = nc.dram_tensor("vT", [128, KC * F], f16, kind="ExternalInput").ap()
    vec_names = []
    vec_d = {}
    for name, used in [
        ("b_v", has_bv), ("b_o", has_bo),
        ("ln_v_g", has_lnv), ("ln_v_b", has_lnv),
        ("ln_o_g", has_lno), ("ln_o_b", has_lno),
        ("ln_ov_g", has_lnov), ("ln_ov_b", has_lnov),
    ]:
        if used:
            vec_d[name] = nc.dram_tensor(name, [D], f32, kind="ExternalInput").ap()
            vec_names.append(name)
    out_d = nc.dram_tensor("out", [F, D], f32, kind="ExternalOutput").ap()

    def bcast_row(vec_ap, parts):
        """AP reading a [D] DRAM vector broadcast over `parts` partitions."""
        return bass.AP(
            tensor=vec_ap.tensor,
            offset=vec_ap.offset,
            ap=[[0, parts]] + list(vec_ap.ap),
        )

    inv_sqrt_d = 1.0 / float(np.sqrt(D))

    with tile.TileContext(nc) as tc:
        with tc.tile_pool(name="persist", bufs=1) as persist, \
             tc.tile_pool(name="stats", bufs=1 if general_B else 2) as stats_pool:

            eps128 = persist.tile([128, 1], f32)
            nc.vector.memset(eps128, LN_EPS)

            bc = {}
            for name in vec_names:
                parts = 128 if name in ("b_o", "ln_o_g", "ln_o_b") else F
                t = persist.tile([parts, D], f32, name=f"bc_{name}", tag=f"bc_{name}")
                nc.gpsimd.dma_start(out=t, in_=bcast_row(vec_d[name], parts))
                bc[name] = t

            def layer_norm_stats(t_in, rows, nsub=ND, width=DW):
                """mean/var over free dim -> mvr tile [:,0]=mu [:,2]=rstd."""
                st = stats_pool.tile([128, nsub, nc.vector.BN_STATS_DIM], f32, tag="st")
                for j in range(nsub):
                    nc.vector.bn_stats(out=st[:rows, j, :],
                                       in_=t_in[:rows, j * width:(j + 1) * width])
                mvr = stats_pool.tile([128, 3], f32, tag="mvr")
                nc.vector.bn_aggr(out=mvr[:rows, 0:2], in_=st[:rows])
                nc.scalar.activation(out=mvr[:rows, 2:3], in_=mvr[:rows, 1:2],
                                     func=AF.Sqrt, bias=eps128[:rows], scale=1.0)
                nc.vector.reciprocal(out=mvr[:rows, 2:3], in_=mvr[:rows, 2:3])
                return mvr

            def apply_ln(t_in, rows, out_tile, mvr, gname, has_g, tn_tile=None):
                """out = (t_in - mu) * rstd [* g + b]; out may cast dtype."""
                if has_g:
                    nc.vector.tensor_scalar(
                        out=tn_tile[:rows], in0=t_in[:rows],
                        scalar1=mvr[:rows, 0:1], scalar2=mvr[:rows, 2:3],
                        op0=OP.subtract, op1=OP.mult)
                    nc.vector.tensor_mul(out=tn_tile[:rows], in0=tn_tile[:rows],
                                         in1=bc[gname + "_g"][:rows])
                    nc.vector.tensor_add(out=out_tile[:rows], in0=tn_tile[:rows],
                                         in1=bc[gname + "_b"][:rows])
                else:
                    nc.vector.tensor_scalar(
                        out=out_tile[:rows], in0=t_in[:rows],
                        scalar1=mvr[:rows, 0:1], scalar2=mvr[:rows, 2:3],
                        op0=OP.subtract, op1=OP.mult)

            ve_nat = persist.tile([F, D], f32)          # LN'd visual embedding
            veT = persist.tile([128, KC, F], f16)       # transposed, for adjacency
            oe_nat = persist.tile([128, NCH, D], f16)   # LN'd object embeddings
            p_nat = persist.tile([128, NCH, F], f16)    # softmax probs, natural
            logits = persist.tile([F, N], f32)          # adj^T logits / probs
            pmax = persist.tile([F, NW + 2], f32)       # per-window max partials

            # W_o tile spans phases A+B; its slices DMA during phase A.
            with tc.tile_pool(name="wo", bufs=1) as wop:
                wo = wop.tile([128, KC * D], f16)

                # ==== Phase A: visual branch (single fp16 pass) ========
                with tc.tile_pool(name="wv", bufs=4) as wvp, \
                     tc.tile_pool(name="vt", bufs=1) as vtp, \
                     tc.tile_pool(name="psA", bufs=1, space="PSUM") as psA, \
                     tc.tile_pool(name="tmpA", bufs=1) as tmpA:
                    vt = vtp.tile([128, KC, F], f16)
                    nc.sync.dma_start(out=vt, in_=vt_d)

                    ps_ve = psA.tile([F, D], f32)
                    for kc in range(KC):
                        wv_k = wvp.tile([128, D], f16, tag="wvk")
                        nc.sync.dma_start(out=wv_k, in_=wv_d[:, kc * D:(kc + 1) * D])
                        # interleave W_o slice loads with phase A's stream
                        nc.sync.dma_start(out=wo[:, kc * D:(kc + 1) * D],
                                          in_=wo_d[:, kc * D:(kc + 1) * D])
                        for dd in range(ND):
                            nc.tensor.matmul(
                                ps_ve[:, dd * DW:(dd + 1) * DW],
                                lhsT=vt[:, kc, :],
                                rhs=wv_k[:, dd * DW:(dd + 1) * DW],
                                start=(kc == 0), stop=(kc == KC - 1))

                    tA = tmpA.tile([F, D], f32)
                    if has_bv:
                        nc.vector.tensor_add(out=tA, in0=ps_ve, in1=bc["b_v"])
                        nc.scalar.activation(out=tA, in_=tA, func=AF.Tanh)
                    else:
                        nc.scalar.activation(out=tA, in_=ps_ve, func=AF.Tanh)
                    mvr = layer_norm_stats(tA, F)
                    tnA = tmpA.tile([F, D], f32) if has_lnv else None
                    apply_ln(tA, F, ve_nat, mvr, "ln_v", has_lnv, tnA)

                    ve_bf = tmpA.tile([F, D], f16)
                    nc.vector.tensor_copy(out=ve_bf, in_=ve_nat)
                    # [64, 2048] -> rows d=(kc*128+kl): [kl, kc, f]
                    nc.sync.dma_start(out=veT, in_=ve_bf, transpose=True)

                # ==== Phase B + fused adjacency (C) ====================
                with tc.tile_pool(name="objs", bufs=2) as objp, \
                     tc.tile_pool(name="psB", bufs=3, space="PSUM") as psB, \
                     tc.tile_pool(name="psC", bufs=2, space="PSUM") as psC, \
                     tc.tile_pool(name="win", bufs=2) as winp, \
                     tc.tile_pool(name="tmpB", bufs=1 if general_B else 2) as tmpB:
                    win_tiles = {}

                    def emit_window_C(w):
                        """Adjacency matmuls for window w (chunks 2w, 2w+1)."""
                        wt = win_tiles.pop(w)
                        padj = psC.tile([F, 256], f32, tag="padj")
                        for kc in range(KC):
                            nc.tensor.matmul(
                                padj,
                                lhsT=veT[:, kc, :],
                                rhs=wt[:, :, kc, :],
                                start=(kc == 0), stop=(kc == KC - 1))
                        nc.scalar.activation(out=logits[:, w * 256:(w + 1) * 256],
                                             in_=padj, func=AF.Copy,
                                             scale=inv_sqrt_d)
                        nc.vector.reduce_max(out=pmax[:, w:w + 1],
                                             in_=logits[:, w * 256:(w + 1) * 256],
                                             axis=AX.X)

                    for nch in range(NCH):
                        objT_nc = objp.tile([128, KC, 128], f16, tag="objT")
                        nc.sync.dma_start(out=objT_nc, in_=objT_d[nch])
                        tB = tmpB.tile([128, D], f32 if general_B else f16, tag="tB")
                        # two half-width PSUM tiles; each half's kc-loop runs
                        # to completion so its tanh overlaps the other half.
                        for h in range(2):
                            ph = psB.tile([128, 2 * DW], f32, tag="psb")
                            for kc in range(KC):
                                for dh in range(2):
                                    dd = 2 * h + dh
                                    nc.tensor.matmul(
                                        ph[:, dh * DW:(dh + 1) * DW],
                                        lhsT=objT_nc[:, kc, :],
                                        rhs=wo[:, kc * D + dd * DW: kc * D + (dd + 1) * DW],
                                        start=(kc == 0), stop=(kc == KC - 1))
                            if has_bo:
                                nc.vector.tensor_add(
                                    out=tB[:, h * 2 * DW:(h + 1) * 2 * DW], in0=ph,
                                    in1=bc["b_o"][:, h * 2 * DW:(h + 1) * 2 * DW])
                                nc.scalar.activation(
                                    out=tB[:, h * 2 * DW:(h + 1) * 2 * DW],
                                    in_=tB[:, h * 2 * DW:(h + 1) * 2 * DW], func=AF.Tanh)
                            else:
                                nc.scalar.activation(
                                    out=tB[:, h * 2 * DW:(h + 1) * 2 * DW],
                                    in_=ph, func=AF.Tanh)
                        mvr = layer_norm_stats(tB, 128)
                        tnB = tmpB.tile([128, D], f32, tag="tnB") if has_lno else None
                        apply_ln(tB, 128, oe_nat[:, nch, :], mvr, "ln_o", has_lno, tnB)

                        # transpose this chunk into its adjacency window
                        w = nch // 2
                        if nch % 2 == 0:
                            win_tiles[w] = winp.tile([128, 2, KC, 128], f16,
                                                     name="winT", tag="winT")
                        nc.sync.dma_start(out=win_tiles[w][:, nch % 2, :, :],
                                          in_=oe_nat[:, nch, :], transpose=True)
                        # emit adjacency work two chunks behind the transposes
                        if nch % 2 == 1 and nch >= 3:
                            emit_window_C((nch - 3) // 2)
                    emit_window_C(NW - 1)

                    # ==== softmax over n (fp32) ========================
                    nc.vector.reduce_max(out=pmax[:, NW:NW + 1],
                                         in_=pmax[:, :NW], axis=AX.X, negate=True)
                    nc.scalar.activation(out=logits, in_=logits, func=AF.Exp,
                                         bias=pmax[:, NW:NW + 1], scale=1.0)
                    nc.vector.reduce_sum(out=pmax[:, NW + 1:NW + 2], in_=logits,
                                         axis=AX.X)
                    nc.vector.reciprocal(out=pmax[:, NW + 1:NW + 2],
                                         in_=pmax[:, NW + 1:NW + 2])
                    p_bf = tmpB.tile([F, N], f16, tag="pbf")
                    nc.vector.tensor_scalar_mul(out=p_bf, in0=logits,
                                                scalar1=pmax[:, NW + 1:NW + 2])
                    # [64, 2304] -> rows n=(ncb*128+nw): [nw, ncb, f]
                    nc.sync.dma_start(out=p_nat, in_=p_bf, transpose=True)

            # ==== Phase D: aggregate + residual + LN ===================
            with tc.tile_pool(name="psD", bufs=1, space="PSUM") as psD, \
                 tc.tile_pool(name="tmpD", bufs=1) as tmpD:
                ps_agg = psD.tile([F, D], f32)
                for nch in range(NCH):
                    for dd in range(ND):
                        nc.tensor.matmul(
                            ps_agg[:, dd * DW:(dd + 1) * DW],
                            lhsT=p_nat[:, nch, :],
                            rhs=oe_nat[:, nch, dd * DW:(dd + 1) * DW],
                            start=(nch == 0), stop=(nch == NCH - 1))

                tD = tmpD.tile([F, D], f32)
                nc.vector.tensor_add(out=tD, in0=ps_agg, in1=ve_nat)
                nc.scalar.activation(out=tD, in_=tD, func=AF.Tanh)
                out_f = tmpD.tile([F, D], f32)
                mvr = layer_norm_stats(tD, F)
                tnD = tmpD.tile([F, D], f32) if has_lnov else None
                apply_ln(tD, F, out_f, mvr, "ln_ov", has_lnov, tnD)
                nc.sync.dma_start(out=out_d, in_=out_f)

    nc.compile()
    _BUILD_CACHE[key] = (nc, vec_names)
    return nc, vec_names


def _prep_core_inputs(visual, obj_flat, shared):
    """Host-side per-sample layout prep. visual [64,2048] f32, obj_flat [2304,2048] f32."""
    m = {
        "objT": np.ascontiguousarray(
            obj_flat.reshape(NCH, 128, KC, 128).transpose(0, 3, 2, 1)
        ).astype(F16).reshape(NCH, 128, KC * 128),
        "vT": np.ascontiguousarray(
            _klc_layout(np.ascontiguousarray(visual.T))).astype(F16),
    }
    m.update(shared)
    return m


def run_kernel(inputs, trace=False):
    """Returns (out [8, 64, 2048] fp32, exec_time_ns or None)."""
    from concourse import bass_utils

    visual = _f32(inputs["visual_feats"])            # [8, 64, 2048]
    obj = _f32(inputs["obj_feats"])                  # [8, 64, 36, 2048]
    W_v = _f32(inputs["W_v"])
    W_o = _f32(inputs["W_o"])
    vecs = {k: _f32(inputs[k]) for k in
            ["b_v", "b_o", "ln_v_g", "ln_v_b", "ln_o_g", "ln_o_b", "ln_ov_g", "ln_ov_b"]}

    has_bv = not np.all(vecs["b_v"] == 0)
    has_bo = not np.all(vecs["b_o"] == 0)
    has_lnv = not (np.all(vecs["ln_v_g"] == 1) and np.all(vecs["ln_v_b"] == 0))
    has_lno = not (np.all(vecs["ln_o_g"] == 1) and np.all(vecs["ln_o_b"] == 0))
    has_lnov = not (np.all(vecs["ln_ov_g"] == 1) and np.all(vecs["ln_ov_b"] == 0))

    nc, vec_names = _build(has_bv, has_bo, has_lnv, has_lno, has_lnov)

    shared = {
        "Wo": np.ascontiguousarray(_klc_layout(W_o)).astype(F16),
        "Wv": np.ascontiguousarray(_klc_layout(W_v)).astype(F16),
    }
    for name in vec_names:
        shared[name] = vecs[name]

    in_maps = [
        _prep_core_inputs(visual[c], obj[c].reshape(N, D), shared)
        for c in range(BS)
    ]

    res = bass_utils.run_bass_kernel_spmd(
        nc, in_maps, core_ids=list(range(BS)), trace=trace)
    out = np.stack([res.results[c]["out"] for c in range(BS)], axis=0)
    return out.astype(np.float32), res.exec_time_ns


def kernel(**inputs):
    out, _ = run_kernel(inputs, trace=False)
    return out
